# revision 1
# baseline (speedup 1.0000x reference)
"""ChebConv (K=2) + temporal Conv1d GNN kernel for 8 Trainium2 NeuronCores.

Strategy (data-parallel over destination nodes):
  - Node axis padded to 50176 = 392 blocks of 128; core c owns blocks
    [49c, 49c+49).
  - Host precomputes w_hat (edge weights of -D^-1/2 A D^-1/2) and sorts the
    edge list by (dst block, src half, dst subblock-of-32), padding each
    group to a multiple of 128 so all 8 cores share one static program.
  - Per block, the device gathers x rows of the edges' sources from an
    fp16 node-major copy of x via SWDGE dma_gather (two calls: src halves,
    since gather indices are int16), builds a sparse "one-hot * w_hat"
    matrix on the fly with broadcast-AP is_equal/mult, and reduces the
    messages with TensorE matmuls into PSUM (segment-sum as matmul).
  - The Chebyshev combine + temporal conv collapse into dense per-node
    matmuls with host-prefolded weights; LeakyReLU finishes on-chip.
"""

import numpy as np

N = 50000
E = 1600000
W = 12
C = 32
WC = W * C            # 384
NCORES = 8
P = 128
NPAD = 50176          # 392 * 128
NB = NPAD // P        # 392
SLOTS = NB // NCORES  # 49
HALF = NPAD // 2      # 25088
NSB = 4               # dst subblocks of 32 per block

_cache = {}


def _host_prep(x, A, Ew):
    src = np.asarray(A[0], np.int64)
    dst = np.asarray(A[1], np.int64)
    Ew = np.asarray(Ew, np.float32)

    deg = np.bincount(dst, weights=Ew.astype(np.float64), minlength=N).astype(np.float32)
    dinv = np.where(deg > 0, 1.0 / np.sqrt(np.maximum(deg, 1e-12)), 0.0).astype(np.float32)
    w_hat = (-dinv[src] * Ew * dinv[dst]).astype(np.float32)

    # node-major x: [NPAD, W*C]
    xrow = np.zeros((NPAD, WC), np.float32)
    xrow[:N] = np.asarray(x, np.float32).transpose(1, 0, 2).reshape(N, WC)
    xrow16 = xrow.astype(np.float16)

    blk = dst >> 7
    sb = (dst >> 5) & 3
    hh = (src >= HALF).astype(np.int64)
    gid = (blk * 2 + hh) * 4 + sb
    order = np.argsort(gid, kind="stable")
    g_sorted = gid[order]
    src_s = src[order]
    dstl_s = (dst[order] & 31).astype(np.float16)
    what_s = w_hat[order].astype(np.float16)
    counts = np.bincount(gid, minlength=NB * 8).reshape(NB, 2, 4)
    gstart = np.zeros(NB * 8 + 1, np.int64)
    np.cumsum(counts.reshape(-1), out=gstart[1:])

    # static chunk counts per (slot, h, s): max over cores
    cnt_c = counts.reshape(NCORES, SLOTS, 2, 4)
    Kg = np.maximum(1, -(-cnt_c // 128)).max(axis=0)  # [SLOTS, 2, 4]
    Jh = Kg.sum(axis=2)                               # [SLOTS, 2]
    Ji = Jh.sum(axis=1)                               # [SLOTS]
    JT = int(Ji.sum())
    IWT = JT * 8

    # column offsets
    joff = np.zeros(SLOTS + 1, np.int64)
    np.cumsum(Ji, out=joff[1:])
    ioff = joff * 8

    idx16 = np.zeros((NCORES, 128, IWT), np.int16)
    dstl_t = np.zeros((NCORES, 128, JT), np.float16)
    what_t = np.zeros((NCORES, 128, JT), np.float16)
    xslot = np.zeros((NCORES, SLOTS * P, WC), np.float32)

    for c in range(NCORES):
        xslot[c] = xrow[c * SLOTS * P:(c + 1) * SLOTS * P]
        for i in range(SLOTS):
            b = c * SLOTS + i
            for h in range(2):
                L = int(Jh[i, h]) * 128
                V = np.zeros(L, np.int16)
                D = np.zeros(L, np.float16)
                Wv = np.zeros(L, np.float16)
                base = 0
                for s in range(4):
                    g = (b * 2 + h) * 4 + s
                    n = int(gstart[g + 1] - gstart[g])
                    sl = slice(int(gstart[g]), int(gstart[g] + n))
                    V[base:base + n] = (src_s[sl] - h * HALF).astype(np.int16)
                    D[base:base + n] = dstl_s[sl]
                    Wv[base:base + n] = what_s[sl]
                    base += int(Kg[i, h, s]) * 128
                co = int(joff[i] + (Jh[i, 0] if h else 0))
                idx_blk = V.reshape(-1, 16).T                    # [16, L/16]
                idx16[c, :, co * 8: co * 8 + L // 16] = np.tile(idx_blk, (8, 1))
                dstl_t[c, :, co: co + L // 128] = D.reshape(-1, 128).T
                what_t[c, :, co: co + L // 128] = Wv.reshape(-1, 128).T

    return xrow16, xslot, idx16, dstl_t, what_t, Kg, Jh, Ji, joff, JT, IWT


def _fold_weights(Wcheb, bcheb, Wconv, bconv):
    Wcheb = np.asarray(Wcheb, np.float32)
    bcheb = np.asarray(bcheb, np.float32)
    Wconv = np.asarray(Wconv, np.float32)
    bconv = np.asarray(bconv, np.float32)
    # pairs (path, gi, go) with |gi-go|<=1
    pairs = []
    for go in range(3):
        for gi in range(max(0, go - 1), min(3, go + 2)):
            for path in range(2):
                pairs.append((path, gi, go))
    mats = np.zeros((len(pairs), 128, 128), np.float32)
    for pi, (path, gi, go) in enumerate(pairs):
        for wo in range(4 * go, 4 * go + 4):
            for k in range(3):
                wi = wo + k - 1
                if not (4 * gi <= wi < 4 * gi + 4) or not (0 <= wi < W):
                    continue
                Cmat = Wcheb[wi, path] @ Wconv[:, :, k].T  # [ci, co]
                r0 = 32 * (wi - 4 * gi)
                c0 = 32 * (wo - 4 * go)
                mats[pi, r0:r0 + 32, c0:c0 + 32] = Cmat
    mats_sb = np.ascontiguousarray(mats.transpose(1, 0, 2).reshape(128, -1))
    bias = np.zeros((12, 32), np.float32)
    for wo in range(12):
        bias[wo] = bconv.copy()
        for k in range(3):
            wi = wo + k - 1
            if 0 <= wi < W:
                bias[wo] += bcheb[wi] @ Wconv[:, :, k].T
    bias_sb = bias.reshape(3, 128).T.copy()  # [128, 3]
    return mats_sb, bias_sb, pairs


def _build_program(Kg, Jh, Ji, joff, JT, IWT, n_pairs):
    import concourse.bacc as bacc
    import concourse.tile as tile
    from concourse import mybir
    import concourse.bass as bass  # noqa

    nc = bacc.Bacc("TRN2", target_bir_lowering=False, debug=False,
                   num_devices=NCORES)
    f16, f32, i16 = mybir.dt.float16, mybir.dt.float32, mybir.dt.int16
    xrow16 = nc.dram_tensor("xrow16", [NPAD, WC], f16, kind="ExternalInput")
    xslot = nc.dram_tensor("xslot", [SLOTS * P, WC], f32, kind="ExternalInput")
    idx16 = nc.dram_tensor("idx16", [128, IWT], i16, kind="ExternalInput")
    dstl = nc.dram_tensor("dstl", [128, JT], f16, kind="ExternalInput")
    what = nc.dram_tensor("what", [128, JT], f16, kind="ExternalInput")
    mats = nc.dram_tensor("mats", [128, n_pairs * 128], f32, kind="ExternalInput")
    biasd = nc.dram_tensor("biasd", [128, 3], f32, kind="ExternalInput")
    iota = nc.dram_tensor("iota", [128, 32], f16, kind="ExternalInput")
    ident = nc.dram_tensor("ident", [128, 128], f32, kind="ExternalInput")
    out_pc = nc.dram_tensor("out_pc", [SLOTS * P, WC], f32, kind="ExternalOutput")

    pairs_by_go = [[], [], []]
    pi = 0
    for go in range(3):
        for gi in range(max(0, go - 1), min(3, go + 2)):
            for path in range(2):
                pairs_by_go[go].append((pi, gi, path))
                pi += 1

    with tile.TileContext(nc) as tc:
        with tc.tile_pool(name="const", bufs=1) as cp, \
             tc.tile_pool(name="sb", bufs=2) as sb, \
             tc.tile_pool(name="xgp", bufs=2) as xgp, \
             tc.tile_pool(name="pst1", bufs=2, space="PSUM") as pst1, \
             tc.tile_pool(name="pstr", bufs=2, space="PSUM") as pstr, \
             tc.tile_pool(name="psy", bufs=2, space="PSUM") as psy:
            mats_t = cp.tile([128, n_pairs * 128], f32)
            nc.sync.dma_start(out=mats_t[:], in_=mats.ap())
            bias_t = cp.tile([128, 3], f32)
            nc.sync.dma_start(out=bias_t[:], in_=biasd.ap())
            iota_t = cp.tile([128, 32], f16)
            nc.sync.dma_start(out=iota_t[:], in_=iota.ap())
            id_t = cp.tile([128, 128], f32)
            nc.sync.dma_start(out=id_t[:], in_=ident.ap())

            import os
            nslots = int(os.environ.get("K_SLOTS", SLOTS))
            sp_flag = os.environ.get("K_SINGLE_PACKET", "0") == "1"
            JMAX = int(Ji.max())
            for i in range(nslots):
                J0, J1 = int(Jh[i, 0]), int(Jh[i, 1])
                J = J0 + J1
                jo = int(joff[i])

                idx_t = sb.tile([128, JMAX * 8], i16, tag="idx")
                nc.sync.dma_start(out=idx_t[:, :J * 8],
                                  in_=idx16.ap()[:, jo * 8:(jo + J) * 8])
                dm_t = sb.tile([128, JMAX], f16, tag="dm")
                nc.sync.dma_start(out=dm_t[:, :J], in_=dstl.ap()[:, jo:jo + J])
                wh_t = sb.tile([128, JMAX], f16, tag="wh")
                nc.sync.dma_start(out=wh_t[:, :J], in_=what.ap()[:, jo:jo + J])

                xg = xgp.tile([128, JMAX, WC], f16, tag="xg")
                nc.gpsimd.dma_gather(
                    xg[:, 0:J0, :], xrow16.ap()[0:HALF, :],
                    idx_t[:, 0:J0 * 8], J0 * 128, J0 * 128, WC,
                    single_packet=sp_flag)
                nc.gpsimd.dma_gather(
                    xg[:, J0:J, :], xrow16.ap()[HALF:NPAD, :],
                    idx_t[:, J0 * 8:J * 8], J1 * 128, J1 * 128, WC,
                    single_packet=sp_flag)

                eq = sb.tile([128, JMAX, 32], f16, tag="eq")
                nc.vector.tensor_tensor(
                    out=eq[:, :J, :],
                    in0=dm_t[:, :J].unsqueeze(2).to_broadcast([128, J, 32]),
                    in1=iota_t[:].unsqueeze(1).to_broadcast([128, J, 32]),
                    op=mybir.AluOpType.is_equal)
                wm = sb.tile([128, JMAX, 32], f16, tag="wm")
                nc.vector.tensor_tensor(
                    out=wm[:, :J, :],
                    in0=eq[:, :J, :],
                    in1=wh_t[:, :J].unsqueeze(2).to_broadcast([128, J, 32]),
                    op=mybir.AluOpType.mult)

                psum_t1 = pst1.tile([128, WC], f32, space="PSUM", tag="t1")
                for s in range(4):
                    first = True
                    for h in range(2):
                        off = (0 if h == 0 else J0) + int(Kg[i, h, :s].sum())
                        for cidx in range(int(Kg[i, h, s])):
                            j = off + cidx
                            last = (h == 1 and cidx == int(Kg[i, 1, s]) - 1)
                            nc.tensor.matmul(
                                out=psum_t1[32 * s:32 * s + 32, :],
                                lhsT=wm[:, j:j + 1, :],
                                rhs=xg[:, j:j + 1, :],
                                start=first, stop=last,
                                tile_position=(0, 32 * s))
                            first = False

                t1sb = sb.tile([128, WC], f32, tag="t1sb")
                nc.scalar.copy(out=t1sb[:], in_=psum_t1[:])
                xb = sb.tile([128, WC], f32, tag="xb")
                nc.sync.dma_start(out=xb[:], in_=xslot.ap()[i * P:(i + 1) * P, :])

                xt = sb.tile([128, WC], f32, tag="xt")
                t1t = sb.tile([128, WC], f32, tag="t1t")
                for t in range(3):
                    ptr = pstr.tile([128, 128], f32, space="PSUM", tag="tr")
                    nc.tensor.transpose(out=ptr[:], in_=xb[:, 128 * t:128 * t + 128],
                                        identity=id_t[:])
                    nc.vector.tensor_copy(out=xt[:, 128 * t:128 * t + 128], in_=ptr[:])
                    ptr2 = pstr.tile([128, 128], f32, space="PSUM", tag="tr")
                    nc.tensor.transpose(out=ptr2[:], in_=t1sb[:, 128 * t:128 * t + 128],
                                        identity=id_t[:])
                    nc.scalar.copy(out=t1t[:, 128 * t:128 * t + 128], in_=ptr2[:])

                yo = sb.tile([128, WC], f32, tag="yo")
                osb = sb.tile([128, WC], f32, tag="osb")
                for go in range(3):
                    py = psy.tile([128, 128], f32, space="PSUM", tag="y")
                    plist = pairs_by_go[go]
                    for n_, (pi_, gi, path) in enumerate(plist):
                        rhs = (xt if path == 0 else t1t)[:, 128 * gi:128 * gi + 128]
                        nc.tensor.matmul(
                            out=py[:], lhsT=mats_t[:, 128 * pi_:128 * pi_ + 128],
                            rhs=rhs, start=(n_ == 0), stop=(n_ == len(plist) - 1),
                            tile_position=(0, 0))
                    ysl = yo[:, 128 * go:128 * go + 128]
                    nc.scalar.activation(out=ysl, in_=py[:],
                                         func=mybir.ActivationFunctionType.Identity,
                                         bias=bias_t[:, go:go + 1], scale=1.0)
                    tl = sb.tile([128, 128], f32, tag="tl")
                    nc.vector.tensor_scalar_mul(out=tl[:], in0=ysl, scalar1=0.01)
                    nc.vector.tensor_tensor(out=ysl, in0=ysl, in1=tl[:],
                                            op=mybir.AluOpType.max)
                    ptr3 = pstr.tile([128, 128], f32, space="PSUM", tag="tr")
                    nc.tensor.transpose(out=ptr3[:], in_=ysl, identity=id_t[:])
                    nc.vector.tensor_copy(out=osb[:, 128 * go:128 * go + 128],
                                          in_=ptr3[:])
                nc.sync.dma_start(out=out_pc.ap()[i * P:(i + 1) * P, :], in_=osb[:])

    nc.compile()
    return nc


def kernel(x, A, Ew, Wcheb, bcheb, Wconv, bconv, batch_size=1):
    from concourse.bass_utils import run_bass_kernel_spmd

    xrow16, xslot, idx16, dstl_t, what_t, Kg, Jh, Ji, joff, JT, IWT = \
        _host_prep(x, A, Ew)
    mats_sb, bias_sb, pairs = _fold_weights(Wcheb, bcheb, Wconv, bconv)

    key = (JT, IWT, tuple(Ji.tolist()))
    if key not in _cache:
        _cache[key] = _build_program(Kg, Jh, Ji, joff, JT, IWT, len(pairs))
    nc = _cache[key]

    iota_np = np.tile(np.arange(32, dtype=np.float16)[None, :], (128, 1))
    ident_np = np.eye(128, dtype=np.float32)
    in_maps = []
    for c in range(NCORES):
        in_maps.append(dict(
            xrow16=xrow16, xslot=xslot[c], idx16=idx16[c],
            dstl=dstl_t[c], what=what_t[c], mats=mats_sb, biasd=bias_sb,
            iota=iota_np, ident=ident_np))
    res = run_bass_kernel_spmd(nc, in_maps, core_ids=list(range(NCORES)))
    full = np.concatenate([res.results[c]["out_pc"] for c in range(NCORES)], axis=0)
    return np.ascontiguousarray(full[:N]).reshape(N, W, C).astype(np.float32)



# revision 8
# speedup vs baseline: 1.4560x; 1.4560x over previous
"""ChebConv (K=2) + temporal Conv1d GNN kernel for 8 Trainium2 NeuronCores.

Strategy (data-parallel over destination nodes):
  - Node axis padded to 50176 = 392 blocks of 128. Blocks are grouped into
    49 slot-groups of 8 (one block per core per slot), matched by per-half
    edge counts (local search) so the shared static program's padded chunk
    counts stay close to each core's real counts.
  - Host precomputes w_hat (edge weights of -D^-1/2 A D^-1/2), sorts each
    (core, slot, src-half) edge group by dst subblock, and pads to a
    multiple of 128 (the padded count = max over the 8 cores).
  - Per slot the device SWDGE-gathers the edges' source rows from an fp8e3
    copy of x padded to 512-B rows (one descriptor per edge at the DMA
    cost model's 512-B sweet spot), builds a weighted one-hot [128, J, 128]
    on DVE, and segment-sums via TensorE with the gathered rows as lhsT so
    each 128-edge chunk costs only span*32 moving rows; the result lands
    feature-major (transposed) which is exactly what the combine needs.
  - Chebyshev combine + temporal conv collapse into 6 dense fp16 matmuls
    with host-prefolded [128, 384] weights + a K=1 bias matmul; LeakyReLU
    runs on the Activation engine; output written fp16 and reassembled on
    host.
"""

import numpy as np
import ml_dtypes

N = 50000
E = 1600000
W = 12
C = 32
WC = W * C            # 384
NCORES = 8
P = 128
NPAD = 50176          # 392 * 128
NB = NPAD // P        # 392
SLOTS = NB // NCORES  # 49
HALF = NPAD // 2      # 25088
ROWE = 512            # gathered row elements (fp8), 384 data + 128 pad
WH_SCALE = 16.0       # fold 1/16 into path-1 combine mats

_cache = {}


def _assign_blocks(cnt):
    """Partition 392 blocks into 49 groups of 8, minimizing
    sum_i sum_h max_c ceil(cnt[g, h]/128)."""
    order = np.argsort(cnt[:, 0], kind="stable")
    groups = order.reshape(SLOTS, NCORES).copy()

    def group_cost(g):
        ch = -(-cnt[g, :] // P)      # [8, 2] ceil
        return int(ch.max(axis=0).sum())

    costs = np.array([group_cost(groups[i]) for i in range(SLOTS)])
    rng = np.random.default_rng(0)
    for _ in range(30000):
        i1, i2 = rng.integers(0, SLOTS, 2)
        if i1 == i2:
            continue
        c1, c2 = rng.integers(0, NCORES, 2)
        g1, g2 = groups[i1].copy(), groups[i2].copy()
        g1[c1], g2[c2] = g2[c2], g1[c1]
        n1, n2 = group_cost(g1), group_cost(g2)
        if n1 + n2 < costs[i1] + costs[i2]:
            groups[i1], groups[i2] = g1, g2
            costs[i1], costs[i2] = n1, n2
    return groups


def _host_prep(x, A, Ew):
    src = np.asarray(A[0], np.int64)
    dst = np.asarray(A[1], np.int64)
    Ew = np.asarray(Ew, np.float32)

    deg = np.bincount(dst, weights=Ew.astype(np.float64), minlength=N).astype(np.float32)
    dinv = np.where(deg > 0, 1.0 / np.sqrt(np.maximum(deg, 1e-12)), 0.0).astype(np.float32)
    w_hat = (-dinv[src] * Ew * dinv[dst]).astype(np.float32)

    # node-major x: fp8e3 gather rows padded to 512 B; fp16 transposed copy
    xr = np.asarray(x, np.float32).transpose(1, 0, 2).reshape(N, WC)
    xrow8 = np.zeros((NPAD, ROWE), ml_dtypes.float8_e3m4)
    xrow8[:N, :WC] = xr.astype(ml_dtypes.float8_e3m4)
    xrow16 = np.zeros((NPAD, WC), np.float16)
    xrow16[:N] = xr.astype(np.float16)

    blk = dst >> 7
    hh = (src >= HALF).astype(np.int64)
    sb = (dst >> 5) & 3
    cnt_bh = np.bincount(blk * 2 + hh, minlength=NB * 2).reshape(NB, 2)

    groups = _assign_blocks(cnt_bh)          # [SLOTS, 8] block ids
    slot_of = np.zeros(NB, np.int64)
    core_of = np.zeros(NB, np.int64)
    for i in range(SLOTS):
        for c in range(NCORES):
            slot_of[groups[i, c]] = i
            core_of[groups[i, c]] = c

    # static chunk counts
    Jh = np.zeros((SLOTS, 2), np.int64)
    for i in range(SLOTS):
        ch = -(-cnt_bh[groups[i]] // P)      # [8, 2]
        Jh[i] = np.maximum(1, ch.max(axis=0))
    Ji = Jh.sum(axis=1)
    JT = int(Ji.sum())
    joff = np.zeros(SLOTS + 1, np.int64)
    np.cumsum(Ji, out=joff[1:])
    IWT = JT * 8

    # sort edges once by (core, slot, h, s)
    gid = ((core_of[blk] * SLOTS + slot_of[blk]) * 2 + hh) * 4 + sb
    order = np.argsort(gid, kind="stable")
    src_s = src[order]
    dstl_s = (dst[order] & 127).astype(np.float16)
    what_s = (w_hat[order] * WH_SCALE).astype(np.float16)
    counts4 = np.bincount(gid, minlength=NB * 8)
    gstart = np.zeros(NB * 8 + 1, np.int64)
    np.cumsum(counts4, out=gstart[1:])

    idx16 = np.zeros((NCORES, 128, IWT), np.int16)
    dstl_t = np.zeros((NCORES, 128, JT), np.float16)
    what_t = np.zeros((NCORES, 128, JT), np.float16)
    xT = np.zeros((NCORES, SLOTS * P, WC), np.float16)
    out_blocks = groups                       # for reassembly

    # spans[i][jj] = sorted list of subblocks present in chunk jj (union over cores)
    span_lo = np.full((SLOTS, int(Ji.max())), 4, np.int64)
    span_hi = np.full((SLOTS, int(Ji.max())), -1, np.int64)

    for i in range(SLOTS):
        J0 = int(Jh[i, 0])
        for c in range(NCORES):
            b = groups[i, c]
            # transposed x for this block: xT[i*128+p, t*128+nn] = x[node nn, feat t*128+p]
            xb = xrow16[b * P:(b + 1) * P, :]              # [128 nodes, 384]
            xT[c, i * P:(i + 1) * P, :] = \
                xb.T.reshape(3, P, P).transpose(1, 0, 2).reshape(P, WC)
            for h in range(2):
                Jg = int(Jh[i, h])
                L = Jg * P
                V = np.zeros(L, np.int16)
                D = np.zeros(L, np.float16)
                Wv = np.zeros(L, np.float16)
                g0 = ((c * SLOTS + i) * 2 + h) * 4
                n = int(gstart[g0 + 4] - gstart[g0])
                sl = slice(int(gstart[g0]), int(gstart[g0] + n))
                V[:n] = (src_s[sl] - h * HALF).astype(np.int16)
                D[:n] = dstl_s[sl]
                Wv[:n] = what_s[sl]
                # per-chunk span of this core (s runs are sorted ascending)
                svals = sb[order][sl]
                co = int(joff[i] + (J0 if h else 0))
                for jj in range(Jg):
                    a0, a1 = jj * P, min(jj * P + P, n)
                    if a0 < a1:
                        smin, smax = int(svals[a0]), int(svals[a1 - 1])
                        gj = co + jj - int(joff[i])
                        span_lo[i, gj] = min(span_lo[i, gj], smin)
                        span_hi[i, gj] = max(span_hi[i, gj], smax)
                idx_blk = V.reshape(-1, 16).T               # [16, L/16]
                idx16[c, :, co * 8: co * 8 + L // 16] = np.tile(idx_blk, (8, 1))
                dstl_t[c, :, co: co + Jg] = D.reshape(-1, P).T
                what_t[c, :, co: co + Jg] = Wv.reshape(-1, P).T

    # build static matmul plan per slot: [(jj, sigma)] in emission order
    plans = []
    for i in range(SLOTS):
        plan = []
        present = set()
        for jj in range(int(Ji[i])):
            lo, hi = int(span_lo[i, jj]), int(span_hi[i, jj])
            if hi < lo:
                lo, hi = 0, 0                      # all-padding chunk
            for s in range(lo, hi + 1):
                plan.append((jj, s))
                present.add(s)
        for s in range(4):
            if s not in present:
                plan.append((0, s))                # zero contribution, defines region
        plans.append(plan)

    return (xrow8, xT, idx16, dstl_t, what_t, Jh, Ji, joff, JT, IWT,
            tuple(tuple(p) for p in map(tuple, plans)), out_blocks)


def _fold_weights(Wcheb, bcheb, Wconv, bconv):
    Wcheb = np.asarray(Wcheb, np.float32)
    bcheb = np.asarray(bcheb, np.float32)
    Wconv = np.asarray(Wconv, np.float32)
    bconv = np.asarray(bconv, np.float32)
    # mats[path, gi]: [128 featin, 384 featout]
    mats = np.zeros((2, 3, P, WC), np.float32)
    for path in range(2):
        for gi in range(3):
            for wl in range(4):
                wi = 4 * gi + wl
                for k in range(3):
                    wo = wi - k + 1
                    if not (0 <= wo < W):
                        continue
                    Cm = Wcheb[wi, path] @ Wconv[:, :, k].T      # [ci, co]
                    mats[path, gi, 32 * wl:32 * wl + 32, 32 * wo:32 * wo + 32] = Cm
    mats[1] /= WH_SCALE
    mats_sb = np.ascontiguousarray(
        mats.reshape(6, P, WC).transpose(1, 0, 2).reshape(P, 6 * WC)).astype(np.float16)
    bias = np.zeros((W, C), np.float32)
    for wo in range(W):
        bias[wo] = bconv.copy()
        for k in range(3):
            wi = wo + k - 1
            if 0 <= wi < W:
                bias[wo] += bcheb[wi] @ Wconv[:, :, k].T
    bias_sb = bias.reshape(1, WC).astype(np.float16)
    return mats_sb, bias_sb


def _build_program(Jh, Ji, joff, JT, IWT, plans):
    import concourse.bacc as bacc
    import concourse.tile as tile
    from concourse import mybir
    import concourse.bass as bass  # noqa

    nc = bacc.Bacc("TRN2", target_bir_lowering=False, debug=False,
                   num_devices=NCORES)
    f16, f32, i16 = mybir.dt.float16, mybir.dt.float32, mybir.dt.int16
    f8 = mybir.dt.float8e3
    xrow8 = nc.dram_tensor("xrow8", [NPAD, ROWE], f8, kind="ExternalInput")
    xTd = nc.dram_tensor("xTd", [SLOTS * P, WC], f16, kind="ExternalInput")
    idx16 = nc.dram_tensor("idx16", [128, IWT], i16, kind="ExternalInput")
    dstl = nc.dram_tensor("dstl", [128, JT], f16, kind="ExternalInput")
    what = nc.dram_tensor("what", [128, JT], f16, kind="ExternalInput")
    mats = nc.dram_tensor("mats", [128, 6 * WC], f16, kind="ExternalInput")
    biasd = nc.dram_tensor("biasd", [1, WC], f16, kind="ExternalInput")
    onesd = nc.dram_tensor("onesd", [1, 128], f16, kind="ExternalInput")
    iota = nc.dram_tensor("iota", [128, 128], f16, kind="ExternalInput")
    out_pc = nc.dram_tensor("out_pc", [SLOTS * P, WC], f16, kind="ExternalOutput")

    JMAX = int(Ji.max())

    with tile.TileContext(nc) as tc:
        with tc.tile_pool(name="const", bufs=1) as cp, \
             tc.tile_pool(name="sb", bufs=2) as sbp, \
             tc.tile_pool(name="xgp", bufs=2) as xgp, \
             tc.tile_pool(name="pst", bufs=2, space="PSUM") as pst, \
             tc.tile_pool(name="psy", bufs=2, space="PSUM") as psy:
            mats_t = cp.tile([128, 6 * WC], f16)
            nc.sync.dma_start(out=mats_t[:], in_=mats.ap())
            bias_t = cp.tile([1, WC], f16)
            nc.sync.dma_start(out=bias_t[:], in_=biasd.ap())
            ones_t = cp.tile([1, 128], f16)
            nc.sync.dma_start(out=ones_t[:], in_=onesd.ap())
            iota_t = cp.tile([128, 128], f16)
            nc.sync.dma_start(out=iota_t[:], in_=iota.ap())

            for i in range(SLOTS):
                J0, J1 = int(Jh[i, 0]), int(Jh[i, 1])
                J = J0 + J1
                jo = int(joff[i])
                plan = plans[i]

                idx_t = sbp.tile([128, JMAX * 8], i16, tag="idx")
                nc.sync.dma_start(out=idx_t[:, :J * 8],
                                  in_=idx16.ap()[:, jo * 8:(jo + J) * 8])
                dm_t = sbp.tile([128, JMAX], f16, tag="dm")
                nc.sync.dma_start(out=dm_t[:, :J], in_=dstl.ap()[:, jo:jo + J])
                wh_t = sbp.tile([128, JMAX], f16, tag="wh")
                nc.sync.dma_start(out=wh_t[:, :J], in_=what.ap()[:, jo:jo + J])

                xg = xgp.tile([128, JMAX, ROWE], f8, tag="xg")
                nc.gpsimd.dma_gather(
                    xg[:, 0:J0, :], xrow8.ap()[0:HALF, :],
                    idx_t[:, 0:J0 * 8], J0 * 128, J0 * 128, ROWE,
                    single_packet=False)
                nc.gpsimd.dma_gather(
                    xg[:, J0:J, :], xrow8.ap()[HALF:NPAD, :],
                    idx_t[:, J0 * 8:J * 8], J1 * 128, J1 * 128, ROWE,
                    single_packet=False)

                eq = sbp.tile([128, JMAX, 128], f16, tag="eq")
                nc.vector.tensor_tensor(
                    out=eq[:, :J, :],
                    in0=dm_t[:, :J].unsqueeze(2).to_broadcast([128, J, 128]),
                    in1=iota_t[:].unsqueeze(1).to_broadcast([128, J, 128]),
                    op=mybir.AluOpType.is_equal)
                wm = sbp.tile([128, JMAX, 128], f8, tag="wm")
                nc.vector.tensor_tensor(
                    out=wm[:, :J, :],
                    in0=eq[:, :J, :],
                    in1=wh_t[:, :J].unsqueeze(2).to_broadcast([128, J, 128]),
                    op=mybir.AluOpType.mult)

                # flipped segment-sum: t1t[t] = [128 feat, 128 dst-in-block]
                # PSUM accumulation groups must be contiguous per bank: each
                # tile gets a full 2KB bank and (t, s) groups are serialized.
                t1p = [pst.tile([128, 512], f32, space="PSUM", tag=f"t1_{t}",
                                name=f"t1p{t}")
                       for t in range(3)]
                by_s = {s: [] for s in range(4)}
                for (jj, s) in plan:
                    by_s[s].append(jj)
                t1s = sbp.tile([128, WC], f16, tag="t1s")
                for t in range(3):
                    for s in range(4):
                        chunks = by_s[s]
                        for k_, jj in enumerate(chunks):
                            nc.tensor.matmul(
                                out=t1p[t][:, 32 * s:32 * s + 32],
                                lhsT=xg[:, jj:jj + 1, 128 * t:128 * t + 128],
                                rhs=wm[:, jj:jj + 1, 32 * s:32 * s + 32],
                                start=(k_ == 0), stop=(k_ == len(chunks) - 1),
                                tile_position=(0, 0))
                    nc.scalar.copy(out=t1s[:, 128 * t:128 * t + 128],
                                   in_=t1p[t][:, 0:128])

                xt = sbp.tile([128, WC], f16, tag="xt")
                nc.sync.dma_start(out=xt[:], in_=xTd.ap()[i * P:(i + 1) * P, :])

                pyt = psy.tile([128, 512], f32, space="PSUM", tag="y")
                nc.tensor.matmul(out=pyt[:, 0:WC], lhsT=ones_t[:], rhs=bias_t[:],
                                 start=True, stop=False, tile_position=(0, 0))
                for path in range(2):
                    srct = xt if path == 0 else t1s
                    for gi in range(3):
                        pi = path * 3 + gi
                        nc.tensor.matmul(
                            out=pyt[:, 0:WC],
                            lhsT=srct[:, 128 * gi:128 * gi + 128],
                            rhs=mats_t[:, pi * WC:(pi + 1) * WC],
                            start=False, stop=(pi == 5),
                            tile_position=(0, 0))

                osb = sbp.tile([128, WC], f16, tag="osb")
                nc.scalar.activation(out=osb[:], in_=pyt[:, 0:WC],
                                     func=mybir.ActivationFunctionType.Lrelu,
                                     bias=0.0, scale=1.0, alpha=0.01)
                nc.sync.dma_start(out=out_pc.ap()[i * P:(i + 1) * P, :], in_=osb[:])

    nc.compile()
    return nc


def kernel(x, A, Ew, Wcheb, bcheb, Wconv, bconv, batch_size=1):
    from concourse.bass_utils import run_bass_kernel_spmd

    (xrow8, xT, idx16, dstl_t, what_t, Jh, Ji, joff, JT, IWT, plans,
     out_blocks) = _host_prep(x, A, Ew)
    mats_sb, bias_sb = _fold_weights(Wcheb, bcheb, Wconv, bconv)

    key = (JT, IWT, plans)
    if key not in _cache:
        _cache[key] = _build_program(Jh, Ji, joff, JT, IWT, plans)
    nc = _cache[key]

    iota_np = np.tile(np.arange(128, dtype=np.float16)[None, :], (128, 1))
    ones_np = np.ones((1, 128), np.float16)
    in_maps = []
    for c in range(NCORES):
        in_maps.append(dict(
            xrow8=xrow8, xTd=xT[c], idx16=idx16[c],
            dstl=dstl_t[c], what=what_t[c], mats=mats_sb, biasd=bias_sb,
            onesd=ones_np, iota=iota_np))
    res = run_bass_kernel_spmd(nc, in_maps, core_ids=list(range(NCORES)))

    out_full = np.zeros((NPAD, WC), np.float32)
    for c in range(NCORES):
        o = np.asarray(res.results[c]["out_pc"], np.float32)
        for i in range(SLOTS):
            b = out_blocks[i, c]
            out_full[b * P:(b + 1) * P] = o[i * P:(i + 1) * P]
    return np.ascontiguousarray(out_full[:N]).reshape(N, W, C)
